# revision 17
# baseline (speedup 1.0000x reference)
"""Trainium2 Bass kernel for nn_DecoderLayer (RNMT+ LN-LSTM decoder layer).

Two-stage pipeline, all device-resident between stages:

  stage A (jax/XLA, shard_map over 8 cores):
    - inputs uploaded as bf16 shards: X batch-sharded, W gate-column-sharded
      (8x less W upload than replication)
    - all_gather W columns, pre-phase GEMM  Z = [x,attn] @ Wx + b  (bf16, fp32
      accumulate), pack Z into the per-step layout the loop kernel wants
  stage B (Bass custom call, per core, batch-data-parallel BL=4):
    - the 256-step recurrence. Per step:
      * 4-way column-tiled GEMM (col-group g <-> gate g, concurrent W streams
        on separate XBUSes) accumulating onto z preloaded in PSUM
      * joint LayerNorm over the (4,1024) gate slab: bn_stats on the PSUM
        banks, indicator-matmul partition combine, fast-inverse-sqrt on DVE
      * ONE sigmoid activation for all 4 gates (tanh via 2*sigmoid(2x)-1,
        the 2x folded into the fisr input for the hid rows)
      * PE-transpose of the activated slab into [osize-part, kk, batch]
        layout so the cheap (FD=32) state update directly yields the next
        step's stationary hT operand
    - epilogue (jax, same jit): transpose h back to [B, S, OSIZE] + residual.

Weights/activations bf16 (matmul), state and LN arithmetic fp32.
"""
import sys

sys.path.insert(0, "/opt/trn_rl_repo")

import numpy as np

import concourse.bass as bass
import concourse.tile as tile
from concourse import bacc, mybir

B, S, ISIZE, OSIZE = 32, 256, 1024, 1024
NCORES = 8
BL = B // NCORES  # 4 batch rows per core
INSZ = ISIZE + OSIZE  # 2048
NG = 4 * OSIZE  # 4096
EPS = 1e-5
F32, BF16, I32 = mybir.dt.float32, mybir.dt.bfloat16, mybir.dt.int32
FISR_MAGIC_F32 = float(
    np.frombuffer(np.array([0x5F3759DF], np.uint32).tobytes(), np.float32)[0]
)
# magic + 0x00800000: seeds rsqrt(v/4) = 2*rsqrt(v) for the tanh-as-2x-sigmoid
# rows (96..127) without an extra scaling op
FISR_MAGIC_HI_F32 = float(
    np.frombuffer(np.array([0x5FB759DF], np.uint32).tobytes(), np.float32)[0]
)

ZCH = 4   # z prefetch chunk (steps per DMA)

_cache = {}


def build_nc(s_steps=S, use_ln=False):
    assert s_steps % ZCH == 0
    nc = bacc.Bacc(None)
    zd = nc.dram_tensor(
        "zd", [s_steps // ZCH, 4, BL, ZCH, OSIZE], BF16, kind="ExternalInput"
    )
    whp = nc.dram_tensor("whp", [128, 8, NG], BF16, kind="ExternalInput")
    ihx = nc.dram_tensor("ihx", [128, 8, BL], BF16, kind="ExternalInput")
    icxT = nc.dram_tensor("icxT", [128, 8, BL], F32, kind="ExternalInput")
    ind = nc.dram_tensor("ind", [128, 128], F32, kind="ExternalInput")
    if use_ln:
        lng = nc.dram_tensor("lng", [128, OSIZE], F32, kind="ExternalInput")
        lnb = nc.dram_tensor("lnb", [128, OSIZE], F32, kind="ExternalInput")
    houts = nc.dram_tensor(
        "houts", [s_steps, 128, 8, BL], BF16, kind="ExternalOutput"
    )

    with tile.TileContext(nc) as tc:
        with (
            tc.tile_pool(name="cw", bufs=1) as cw,
            tc.tile_pool(name="sp", bufs=2) as sp,
            tc.tile_pool(name="hp", bufs=2) as hp,
            tc.tile_pool(name="yp", bufs=2) as yp,
            tc.tile_pool(name="htp", bufs=2) as htp,
            tc.tile_pool(name="gzp", bufs=1, space="PSUM") as gzp,
            tc.tile_pool(name="ytp", bufs=2, space="PSUM") as ytp,
            tc.tile_pool(name="sps", bufs=1, space="PSUM") as sps,
        ):
            whs = cw.tile([128, 8, NG], BF16)
            nc.sync.dma_start(out=whs, in_=whp[:, :, :])
            inds = cw.tile([128, 128], F32)
            nc.sync.dma_start(out=inds, in_=ind[:, :])
            if use_ln:
                lngs = cw.tile([128, OSIZE], F32)
                nc.sync.dma_start(out=lngs, in_=lng[:, :])
                lnbs = cw.tile([128, OSIZE], F32)
                nc.sync.dma_start(out=lnbs, in_=lnb[:, :])
            magic = cw.tile([128, 1], F32)
            nc.vector.memset(magic[0:96, :], FISR_MAGIC_F32)
            nc.vector.memset(magic[96:128, :], FISR_MAGIC_HI_F32)
            coef = cw.tile([128, 1], F32)  # Newton -0.5 (x4 for tanh rows)
            nc.vector.memset(coef[0:96, :], -0.5)
            nc.vector.memset(coef[96:128, :], -0.125)
            scrA = cw.tile([128, 512], BF16)  # ttr elementwise dump (unused)
            scrB = cw.tile([128, 512], BF16)  # ACT square dump (unused)
            zrs = cw.tile([128, 512], BF16)  # zeros (ttr second operand)
            nc.vector.memset(zrs, 0.0)
            id128 = cw.tile([128, 128], BF16)
            from concourse.masks import make_identity

            make_identity(nc, id128)

            c = cw.tile([128, 8, BL], F32)  # cell state, transposed layout
            nc.sync.dma_start(out=c, in_=icxT[:, :, :])
            hT = htp.tile([128, 8, BL], BF16, tag="hT")
            nc.sync.dma_start(out=hT, in_=ihx[:, :, :])

            # gate-slab PSUM accumulator [128, 1024] f32 = 2 banks; col-group
            # g rows 32g..32g+BL-1 carry gate g, cols = osize. Rows 32g+BL..
            # 32g+31 stay 0 forever (memset once; matmuls never touch them,
            # the per-step z preload copies zeros there).
            gz = gzp.tile([128, OSIZE], F32)
            nc.vector.memset(gz, 0.0)
            # z double buffers: rows beyond the BL valid ones per gate group
            # stay 0 so full-width psum preloads are safe
            z4bufs = [
                cw.tile([128, ZCH, OSIZE], BF16, name=f"z4b{i}", tag=f"z4b{i}")
                for i in range(2)
            ]
            for zb in z4bufs:
                nc.vector.memset(zb, 0.0)

            for t in range(s_steps):
                tz = t % ZCH
                if t == 0:
                    z4 = z4bufs[0]
                    for zg in range(4):
                        nc.sync.dma_start(
                            out=z4[32 * zg : 32 * zg + BL, :, :],
                            in_=zd[0, zg, :, :, :],
                        )
                else:
                    z4 = z4bufs[(t // ZCH) % 2]

                # ---- g = z_t + h @ Wh  (z preloaded in PSUM, except t=0) ----
                for h2 in range(2):
                    cs = slice(h2 * 512, h2 * 512 + 512)
                    for kk in range(8):
                        for g4 in range(4):
                            nc.tensor.matmul(
                                gz[32 * g4 : 32 * g4 + BL, cs],
                                hT[:, kk, :],
                                whs[:, kk, g4 * 1024 + h2 * 512 : g4 * 1024 + h2 * 512 + 512],
                                start=(t == 0 and kk == 0),
                                stop=(kk == 7),
                                tile_position=(0, 32 * g4),
                                skip_group_check=True,
                            )
                    if t == 0:
                        nc.vector.tensor_add(gz[:, cs], gz[:, cs], z4[:, tz, cs])

                # ---- joint LN stats over the gate slab (all on ACT) ----
                # mv cols: 0,1 = -sum(g)/4096 per half (Copy + accum),
                #          2,3 = sum(g^2)/4096 per half (Square + accum)
                mv = sp.tile([128, 4], F32, tag="mv")
                for h2 in range(2):
                    cs = slice(h2 * 512, h2 * 512 + 512)
                    nc.scalar.activation(
                        out=scrA, in_=gz[:, cs],
                        func=mybir.ActivationFunctionType.Copy,
                        scale=-1.0 / NG, accum_out=mv[:, h2 : h2 + 1],
                    )
                    nc.scalar.activation(
                        out=scrB, in_=gz[:, cs],
                        func=mybir.ActivationFunctionType.Square,
                        scale=1.0 / 64.0, accum_out=mv[:, 2 + h2 : 3 + h2],
                    )
                # combine the 4 gate groups + broadcast: pss[p] cols =
                # [-mean_h0, -mean_h1, E_h0, E_h1] for batch row p%32
                pss = sps.tile([128, 4], F32, tag="pss")
                nc.tensor.matmul(pss, inds, mv, start=True, stop=True)
                q = sp.tile([128, 4], F32, tag="q")
                nc.vector.tensor_copy(q, pss)
                meanq = sp.tile([128, 1], F32, tag="meanq")  # = -mean
                nc.vector.tensor_add(meanq, q[:, 0:1], q[:, 1:2])
                vh = sp.tile([128, 1], F32, tag="vh")
                # vh = mean^2 - E_h0
                nc.vector.scalar_tensor_tensor(
                    out=vh, in0=meanq, scalar=meanq,
                    in1=q[:, 2:3],
                    op0=mybir.AluOpType.mult, op1=mybir.AluOpType.subtract,
                )
                vhp = sp.tile([128, 1], F32, tag="vhp")
                nc.vector.tensor_sub(vhp, q[:, 3:4], vh)  # = var
                # fast inverse sqrt + 1 Newton iteration -> rstd
                # (magic/coef rows 96..127 fold in the x2 for tanh-as-sigmoid)
                ish = sp.tile([128, 1], I32, tag="ish")
                nc.vector.tensor_scalar(
                    out=ish, in0=vhp.bitcast(I32), scalar1=1, scalar2=None,
                    op0=mybir.AluOpType.logical_shift_right,
                )
                y0 = sp.tile([128, 1], F32, tag="y0")
                nc.vector.tensor_sub(y0.bitcast(I32), magic.bitcast(I32), ish)
                t2 = sp.tile([128, 1], F32, tag="t2")
                nc.vector.scalar_tensor_tensor(
                    out=t2, in0=y0, scalar=y0, in1=vhp,
                    op0=mybir.AluOpType.mult, op1=mybir.AluOpType.mult,
                )
                nc.vector.tensor_scalar(
                    out=t2, in0=t2, scalar1=coef, scalar2=1.5,
                    op0=mybir.AluOpType.mult, op1=mybir.AluOpType.add,
                )
                rstd = sp.tile([128, 1], F32, tag="rstd")
                nc.vector.tensor_mul(rstd, y0, t2)
                nbias = sp.tile([128, 1], F32, tag="nbias")  # -mu*rstd
                nc.vector.tensor_mul(nbias, meanq, rstd)

                # ---- normalize + activate: ONE sigmoid over the whole slab
                y = yp.tile([128, OSIZE], BF16, tag="y")
                if use_ln:
                    y2n = yp.tile([128, OSIZE], F32, tag="y2n")
                    nc.vector.tensor_scalar(
                        out=y2n, in0=gz, scalar1=rstd, scalar2=nbias,
                        op0=mybir.AluOpType.mult, op1=mybir.AluOpType.add,
                    )
                    nc.vector.tensor_mul(y2n, y2n, lngs)
                    nc.vector.tensor_add(y2n, y2n, lnbs)
                    nc.scalar.activation(
                        out=y[0:96, :], in_=y2n[0:96, :],
                        func=mybir.ActivationFunctionType.Sigmoid,
                    )
                    nc.scalar.activation(
                        out=y[96:128, :], in_=y2n[96:128, :],
                        func=mybir.ActivationFunctionType.Sigmoid, scale=2.0,
                    )
                else:
                    nc.scalar.activation(
                        out=y, in_=gz,
                        func=mybir.ActivationFunctionType.Sigmoid,
                        bias=nbias, scale=rstd,
                    )

                # preload next step's z into the PSUM banks on ACT (in-order
                # after sigmoid has consumed them); matmul start=False then
                # accumulates onto it
                if t + 1 < s_steps:
                    tn = t + 1
                    z4n = z4bufs[(tn // ZCH) % 2]
                    if tn % ZCH == 0:
                        for zg in range(4):
                            nc.sync.dma_start(
                                out=z4n[32 * zg : 32 * zg + BL, :, :],
                                in_=zd[tn // ZCH, zg, :, :, :],
                            )
                    for h2 in range(2):
                        cs = slice(h2 * 512, h2 * 512 + 512)
                        nc.scalar.activation(
                            out=gz[:, cs], in_=z4n[:, tn % ZCH, cs],
                            func=mybir.ActivationFunctionType.Copy,
                        )

                # ---- transpose the activated slab: yT[c, j, r] = y[r, 128j+c]
                yT = ytp.tile([128, 8, 128], BF16, tag="yT")
                for j in range(8):
                    nc.tensor.transpose(
                        yT[:, j, :], y[:, 128 * j : 128 * j + 128], id128
                    )

                # ---- state update in transposed layout (FD=32 ops) ----
                hid = hp.tile([128, 8, BL], F32, tag="hid")  # 2*s-1
                nc.vector.tensor_scalar(
                    out=hid, in0=yT[:, :, 96 : 96 + BL], scalar1=2.0,
                    scalar2=-1.0, op0=mybir.AluOpType.mult,
                    op1=mybir.AluOpType.add,
                )
                u = hp.tile([128, 8, BL], F32, tag="u")
                nc.vector.tensor_mul(u, yT[:, :, 0:BL], hid)  # i*hid
                t1 = hp.tile([128, 8, BL], F32, tag="t1")
                nc.vector.tensor_mul(t1, yT[:, :, 32 : 32 + BL], c)  # f*c
                nc.vector.tensor_add(c, t1, u)  # c = f*c + i*hid
                hTb = htp.tile([128, 8, BL], BF16, tag="hT")
                nc.vector.tensor_mul(hTb, yT[:, :, 64 : 64 + BL], c)  # h = o*c
                nc.gpsimd.dma_start(out=houts[t, :, :, :], in_=hTb)
                hT = hTb
    nc.finalize()
    return nc



# ---------------------------------------------------------------------------
# host prep + jax pipeline (stage A: gathers + pre-phase GEMM; stage B: bass
# recurrence + epilogue transpose/residual)
# ---------------------------------------------------------------------------
import ml_dtypes

BF16NP = ml_dtypes.bfloat16


def _to_bf16(a):
    """fp32 -> bf16 with round-to-nearest-even via integer view (fast)."""
    u = np.ascontiguousarray(a, np.float32).view(np.uint32)
    r = ((u + np.uint32(0x7FFF) + ((u >> np.uint32(16)) & np.uint32(1)))
         >> np.uint32(16)).astype(np.uint16)
    return r.view(BF16NP)


_pp = np.arange(128)
IND_NP = (
    (_pp[:, None] % 32 == _pp[None, :] % 32) & (_pp[:, None] % 32 < 4)
).astype(np.float32)


def _build_pipeline(s_steps, use_ln):
    """Returns (run, put) where put(host arrays)->device arrays and
    run(dev)->jax out array [B, s, OSIZE]."""
    import jax
    import jax.numpy as jnp
    from jax.sharding import Mesh, PartitionSpec as P, NamedSharding
    from jax.experimental.shard_map import shard_map
    from concourse.bass2jax import (
        install_neuronx_cc_hook,
        partition_id_tensor,
        _bass_exec_p,
    )

    install_neuronx_cc_hook()
    nc = build_nc(s_steps, use_ln)

    devices = jax.devices()[:NCORES]
    mesh = Mesh(np.asarray(devices), ("c",))

    # ---- stage B: bass custom call + epilogue ----
    partition_name = nc.partition_id_tensor.name if nc.partition_id_tensor else None
    in_names, out_names, out_avals = [], [], []
    for alloc in nc.m.functions[0].allocations:
        if not isinstance(alloc, mybir.MemoryLocationSet):
            continue
        name = alloc.memorylocations[0].name
        if alloc.kind == "ExternalInput":
            if name != partition_name:
                in_names.append(name)
        elif alloc.kind == "ExternalOutput":
            out_names.append(name)
            import jax.core

            out_avals.append(
                jax.core.ShapedArray(tuple(alloc.tensor_shape), mybir.dt.np(alloc.dtype))
            )
    all_in = in_names + out_names + ([partition_name] if partition_name else [])

    def _bass_body(*args):
        operands = list(args)
        if partition_name is not None:
            operands.append(partition_id_tensor())
        outs = _bass_exec_p.bind(
            *operands,
            out_avals=tuple(out_avals),
            in_names=tuple(all_in),
            out_names=tuple(out_names),
            lowering_input_output_aliases=(),
            sim_require_finite=True,
            sim_require_nnan=True,
            nc=nc,
        )
        return tuple(outs)

    n_bass_in = len(in_names)
    bass_specs = (P("c"),) * (n_bass_in + len(out_names))
    stageB = jax.jit(
        shard_map(
            _bass_body, mesh=mesh, in_specs=bass_specs,
            out_specs=(P("c"),) * len(out_names), check_rep=False,
        ),
        keep_unused=True,
    )

    # ---- stage C: epilogue (transpose back + residual) ----
    def _epi_body(houts, xo):
        # houts: [s, 128, 8, BL] bf16 per core; xo: [BL, s, OSIZE] bf16
        h = jnp.transpose(houts, (3, 0, 2, 1)).reshape(BL, s_steps, OSIZE)
        return (h.astype(jnp.float32) + xo.astype(jnp.float32),)

    stageC = jax.jit(
        shard_map(
            _epi_body, mesh=mesh, in_specs=(P("c"), P("c")),
            out_specs=(P("c"),), check_rep=False,
        )
    )

    # ---- stage A: pure jax ----
    def _prep_body(xo, xa, wx, wh, bvec, ihx0, icx0, indr, lng, lnb):
        # xo, xa: [BL, s, 1024] bf16 (per core batch slice)
        # wx: [2048, 512] bf16 (per core gate-column slab), wh: [1024, 512]
        Wx = jax.lax.all_gather(wx, "c", axis=1, tiled=True)  # [2048, 4096]
        Wh = jax.lax.all_gather(wh, "c", axis=1, tiled=True)  # [1024, 4096]
        X2 = jnp.concatenate([xo, xa], axis=-1).reshape(BL * s_steps, INSZ)
        Z = (
            jnp.dot(X2, Wx, preferred_element_type=jnp.float32)
            + bvec[None, :]
        )
        zdl = (
            Z.reshape(BL, s_steps // ZCH, ZCH, 4, OSIZE)
            .astype(jnp.bfloat16)
            .transpose(1, 3, 0, 2, 4)
            .reshape(s_steps // ZCH, 4, BL, ZCH, OSIZE)
        )
        whpl = Wh.reshape(8, 128, NG).transpose(1, 0, 2)  # [128, 8, NG] bf16
        ihxT = jnp.broadcast_to(
            ihx0.reshape(8, 128).T[:, :, None], (128, 8, BL)
        ).astype(jnp.bfloat16)
        icxT = jnp.broadcast_to(
            icx0.reshape(8, 128).T[:, :, None], (128, 8, BL)
        ).astype(jnp.float32)
        outs = [zdl, whpl, ihxT, icxT, indr]
        if use_ln:
            l16g = jnp.repeat(lng, 32, axis=0)  # [4,1024]->[128,1024], row 32g+b
            l16b = jnp.repeat(lnb, 32, axis=0)
            outs += [l16g, l16b]
        outs.append(
            jnp.zeros((s_steps, 128, 8, BL), jnp.bfloat16)
        )  # houts buffer
        return tuple(outs)

    a_in = (P("c"), P("c"), P(None, "c"), P(None, "c"), P(), P(), P(), P(), P(), P())
    a_out = (P("c"),) * (n_bass_in + 1)
    stageA = jax.jit(
        shard_map(_prep_body, mesh=mesh, in_specs=a_in, out_specs=a_out,
                  check_rep=False)
    )

    sh_b = NamedSharding(mesh, P("c"))
    sh_w = NamedSharding(mesh, P(None, "c"))
    sh_r = NamedSharding(mesh, P())

    def put(inputo, attn, W, bvec, ln_g, ln_b, init_hx, init_cx):
        import jax
        from concurrent.futures import ThreadPoolExecutor

        with ThreadPoolExecutor(3) as ex:
            fxo = ex.submit(lambda: _to_bf16(np.asarray(inputo)[:, :s_steps]))
            fxa = ex.submit(lambda: _to_bf16(np.asarray(attn)[:, :s_steps]))
            fwb = ex.submit(lambda: _to_bf16(np.asarray(W)))
            xo, xa, Wb = fxo.result(), fxa.result(), fwb.result()
        dev = dict(
            xo=jax.device_put(xo, sh_b),
            xa=jax.device_put(xa, sh_b),
            wx=jax.device_put(Wb[:INSZ], sh_w),
            wh=jax.device_put(Wb[INSZ:], sh_w),
            bvec=jax.device_put(np.asarray(bvec, np.float32), sh_r),
            ihx0=jax.device_put(
                np.asarray(init_hx, np.float32).reshape(OSIZE), sh_r
            ),
            icx0=jax.device_put(
                np.asarray(init_cx, np.float32).reshape(OSIZE), sh_r
            ),
            indr=jax.device_put(IND_NP, sh_r),
            lng=jax.device_put(np.asarray(ln_g, np.float32), sh_r),
            lnb=jax.device_put(np.asarray(ln_b, np.float32), sh_r),
        )
        return dev

    def run(dev):
        pre = stageA(
            dev["xo"], dev["xa"], dev["wx"], dev["wh"], dev["bvec"],
            dev["ihx0"], dev["icx0"], dev["indr"], dev["lng"], dev["lnb"],
        )
        outs = stageB(*pre)
        return stageC(outs[0], dev["xo"])[0]

    _dbg[(s_steps, use_ln)] = dict(stageA=stageA, stageB=stageB, stageC=stageC, nc=nc)
    return run, put


_dbg = {}


def _get_pipeline(s_steps, use_ln):
    key = (s_steps, use_ln)
    if key not in _cache:
        _cache[key] = _build_pipeline(s_steps, use_ln)
    return _cache[key]


def kernel(inputo, attn, W, b, ln_g, ln_b, init_hx, init_cx):
    import jax

    ln_g = np.asarray(ln_g, np.float32)
    ln_b = np.asarray(ln_b, np.float32)
    use_ln = not (np.all(ln_g == 1.0) and np.all(ln_b == 0.0))
    run, put = _get_pipeline(S, use_ln)
    dev = put(inputo, attn, W, b, ln_g, ln_b, init_hx, init_cx)
    out = run(dev)
    return np.asarray(out)
